# revision 14
# baseline (speedup 1.0000x reference)
"""FeatureVarianceLoss Trainium2 kernel.

Math (per keypoint n; V=16 vectors of C=256 channels):
    x_hat = x / ||x||                       (L2 normalize over C)
    pair_sum = V * sum||x_hat||^2 - ||sum_v x_hat||^2
    var_gt = max(pair_sum / (V*(V-1)/2), 0.05)
    vp     = mean_v(var_pred) + 1e-6
    loss   = mean_n |ln(vp) - ln(var_gt)|

sum_v||x_hat||^2 == V to ~1e-7 relative (norms are ~16, never near the 1e-12
clamp for randn inputs), so pair_sum = V^2 - ||s||^2 with s = sum_v x/||x||.

Sharding: data-parallel over n, 1024 keypoints per core across 8 cores.
Each core outputs [128,1] partial sums of |ln vp - ln var_gt|; the host
sums them and divides by N. Inputs are pre-swizzled on the host:
  desc  [1024, 4096] bf16: [g*128+p, j*256+c] = flat row 2048g + 128j + p
        of the shard's [NS*V, C] view (each group g is one contiguous 1MB
        DMA whose SBUF layout is [p, j, c] subtiles)
  vpred [128, 8*16] f32:   [p, g*16+v] = var_pred[.. + g*128 + p, v]
  maskin [128, 4, 32] f32: block-diagonal selection masks

Per-core pipeline: 8 groups of 128 keypoints; a group is 16 subtiles of
[128 rows=(n,v), 256 C]. Engines:
  ACT: Square pass (norms), Sqrt, ||s||^2 via Square+accum_out, final Ln
  DVE: segmented reduce over C, reciprocal + one Newton rsqrt polish,
       weight build (mask * inv), epilogue
  PE : s = sum_v x * inv via 16 block-diagonal-masked bf16 matmuls per
       group (M=32 col-tiles accumulated into one PSUM [128,256] tile)
"""

import ml_dtypes
import numpy as np

N_FULL, V, C = 8192, 16, 256
NCORES = 8
NS = N_FULL // NCORES  # 1024 keypoints per core
GROUPS = NS // 128     # 8
SUBT = 16              # subtiles per group, each [128, 256]
KH = 8                 # subtiles whose C-halving is offloaded to GPSIMD
EPS = 1e-6
VAR_CLAMP = 0.05
PAIR_CNT = V * (V - 1) // 2  # 120


def build_nc(use_tt_broadcast_weights=True):
    from contextlib import ExitStack

    import concourse.bass as bass
    import concourse.mybir as mybir
    from concourse import bacc, tile

    f32 = mybir.dt.float32
    bf16 = mybir.dt.bfloat16
    AF = mybir.ActivationFunctionType
    ALU = mybir.AluOpType
    AX = mybir.AxisListType.X

    nc = bacc.Bacc()
    desc = nc.declare_dram_parameter("desc", [GROUPS * 128, SUBT * C], bf16, isOutput=False)
    vpred = nc.declare_dram_parameter("vpred", [128, GROUPS * V], f32, isOutput=False)
    maskin = nc.declare_dram_parameter("maskin", [128, 4, 32], f32, isOutput=False)
    out = nc.declare_dram_parameter("out", [128, 1], f32, isOutput=True)

    with tile.TileContext(nc) as tc, ExitStack() as ctx:
        xpool = ctx.enter_context(tc.tile_pool(name="x", bufs=GROUPS))
        sqpool = ctx.enter_context(tc.tile_pool(name="sq", bufs=3))
        wpool = ctx.enter_context(tc.tile_pool(name="w", bufs=3))
        tmp = ctx.enter_context(tc.tile_pool(name="tmp", bufs=8))
        persist = ctx.enter_context(tc.tile_pool(name="persist", bufs=1))
        psum = ctx.enter_context(
            tc.tile_pool(name="psum", bufs=4, space=bass.MemorySpace.PSUM)
        )

        # Block-diagonal selection masks (host-built): mask[p, r, m] = 1 iff
        # m == 8r + p//16. Subtile j writes PSUM partitions [32*(j//4), +32)
        # with its 8 keypoints at column offset 8*(j%4) within the block.
        masks = persist.tile([128, 4, 32], f32, tag="masks")
        nc.sync.dma_start(out=masks[:], in_=maskin[:])

        # All of var_pred in one DMA + one segmented reduce: vps_all[p, g]
        vt = persist.tile([128, GROUPS, V], f32, tag="vt")
        nc.sync.dma_start(out=vt[:], in_=vpred[:].rearrange("p (g v) -> p g v", v=V))
        vps_all = persist.tile([128, GROUPS], f32, tag="vps_all")
        nc.vector.reduce_sum(out=vps_all[:], in_=vt[:], axis=AX)

        s2_all = persist.tile([128, GROUPS], f32, tag="s2_all")
        eps_ap = persist.tile([128, 1], f32, tag="eps")
        nc.vector.memset(eps_ap[:], EPS)

        for g in range(GROUPS):
            # One contiguous 1MB DMA per group; SBUF layout [p, j, c]
            x = xpool.tile([128, SUBT, C], bf16, tag="x")
            nc.sync.dma_start(
                out=x[:],
                in_=desc[128 * g : 128 * (g + 1), :].rearrange("p (j c) -> p j c", c=C),
            )

            # norm^2 per (n, v) row. GPSIMD (otherwise idle) pre-adds the
            # two C-halves for the first KH subtiles to offload the DVE
            # reduce; DVE reduces the halved tiles + the remaining subtiles.
            sq = sqpool.tile([128, SUBT, C], f32, tag="sq")
            nc.scalar.activation(sq[:], x[:], AF.Square)
            half = sqpool.tile([128, KH, C // 2], f32, tag="half")
            nc.gpsimd.tensor_tensor(
                out=half[:],
                in0=sq[:, :KH, : C // 2],
                in1=sq[:, :KH, C // 2 :],
                op=ALU.add,
            )
            norm2 = tmp.tile([128, SUBT], f32, tag="norm2")
            nc.vector.reduce_sum(out=norm2[:, :KH], in_=half[:], axis=AX)
            nc.vector.reduce_sum(out=norm2[:, KH:], in_=sq[:, KH:, :], axis=AX)

            # inv = rsqrt(norm2): ACT Sqrt (loose ULP budget) + DVE reciprocal,
            # then one Newton step  inv <- inv * (1.5 - 0.5 * norm2 * inv^2)
            norm = tmp.tile([128, SUBT], f32, tag="norm")
            nc.scalar.sqrt(norm[:], norm2[:])
            inv0 = tmp.tile([128, SUBT], f32, tag="inv0")
            nc.vector.reciprocal(inv0[:], norm[:])
            t1 = tmp.tile([128, SUBT], f32, tag="t1")
            nc.vector.tensor_mul(t1[:], inv0[:], inv0[:])
            nc.vector.tensor_mul(t1[:], t1[:], norm2[:])
            nc.vector.tensor_scalar(t1[:], t1[:], -0.5, 1.5, ALU.mult, ALU.add)
            inv = tmp.tile([128, SUBT], f32, tag="inv")
            nc.vector.tensor_mul(inv[:], inv0[:], t1[:])

            # weights w[p, j, m] = mask[p, j%4, m] * inv[p, j]
            w = wpool.tile([128, SUBT, 32], bf16, tag="w")
            if use_tt_broadcast_weights:
                m_b = masks[:].unsqueeze(1).broadcast_to((128, 4, 4, 32))
                i_b = (
                    inv[:]
                    .rearrange("p (jj r) -> p jj r", r=4)
                    .unsqueeze(3)
                    .broadcast_to((128, 4, 4, 32))
                )
                nc.vector.tensor_tensor(
                    out=w[:].rearrange("p (jj r) m -> p jj r m", r=4),
                    in0=m_b,
                    in1=i_b,
                    op=ALU.mult,
                )
            else:
                for j in range(SUBT):
                    nc.vector.tensor_scalar_mul(
                        w[:, j, :], masks[:, j % 4, :], inv[:, j : j + 1]
                    )

            # s[n, c] = sum_v x * inv, 16 matmuls accumulating into one PSUM tile
            ps = psum.tile([128, C], f32, tag="ps")
            for b in range(4):
                for r in range(4):
                    j = 4 * b + r
                    nc.tensor.matmul(
                        ps[32 * b : 32 * b + 32, :],
                        w[:, j, :],
                        x[:, j, :],
                        start=(r == 0),
                        stop=(r == 3),
                        tile_position=(0, 32 * b),
                    )

            # ||s||^2 per keypoint
            s2sc = tmp.tile([128, C], f32, tag="s2sc")
            nc.scalar.activation(
                s2sc[:], ps[:], AF.Square, accum_out=s2_all[:, g : g + 1]
            )

        # Epilogue over all groups at once ([128, 8] tiles)
        logvp = persist.tile([128, GROUPS], f32, tag="logvp")
        nc.scalar.activation(logvp[:], vps_all[:], AF.Ln, bias=eps_ap[:], scale=1.0 / V)

        pg = persist.tile([128, GROUPS], f32, tag="pg")
        nc.vector.tensor_scalar(
            pg[:], s2_all[:], -1.0 / PAIR_CNT, float(V * V) / PAIR_CNT, ALU.mult, ALU.add
        )
        nc.vector.tensor_scalar_max(pg[:], pg[:], VAR_CLAMP)
        loggt = persist.tile([128, GROUPS], f32, tag="loggt")
        nc.scalar.activation(loggt[:], pg[:], AF.Ln)

        diff = persist.tile([128, GROUPS], f32, tag="diff")
        nc.vector.tensor_sub(diff[:], logvp[:], loggt[:])
        acc = persist.tile([128, 1], f32, tag="acc")
        nc.vector.tensor_reduce(
            out=acc[:], in_=diff[:], axis=AX, op=ALU.add, apply_absolute_value=True
        )
        nc.sync.dma_start(out=out[:], in_=acc[:])

    nc.finalize()
    return nc


def host_masks():
    m = np.zeros((128, 4, 32), dtype=np.float32)
    p = np.arange(128)
    for r in range(4):
        m[p, r, 8 * r + p // 16] = 1.0
    return m


def swizzle_desc(dshard):
    # dshard [NS*V, C] fp32 -> [1024, 4096] bf16 with row g*128+p holding
    # subtiles [j, c] = flat row 2048g + 128j + p
    d = dshard.reshape(GROUPS, SUBT, 128, C)
    d = d.transpose(0, 2, 1, 3).reshape(GROUPS * 128, SUBT * C)
    return np.ascontiguousarray(d.astype(ml_dtypes.bfloat16))


def swizzle_vpred(vshard):
    # vshard [NS, V] fp32 -> [128, GROUPS*V] with [p, g*16+v] = row 128g+p
    v = vshard.reshape(GROUPS, 128, V).transpose(1, 0, 2).reshape(128, GROUPS * V)
    return np.ascontiguousarray(v.astype(np.float32))


def make_in_maps(desc_var, var_pred):
    mask = host_masks()
    in_maps = []
    for c in range(NCORES):
        dshard = desc_var[c * NS : (c + 1) * NS].reshape(NS * V, C)
        vshard = var_pred[c * NS : (c + 1) * NS, :, 0]
        in_maps.append(
            {
                "desc": swizzle_desc(dshard),
                "vpred": swizzle_vpred(vshard),
                "maskin": mask,
            }
        )
    return in_maps


def kernel(desc_var, var_pred):
    from concourse.bass_utils import run_bass_kernel_spmd

    desc_var = np.asarray(desc_var, dtype=np.float32)
    var_pred = np.asarray(var_pred, dtype=np.float32)
    nc = build_nc()
    res = run_bass_kernel_spmd(nc, make_in_maps(desc_var, var_pred), list(range(NCORES)))
    total = sum(float(r["out"].sum()) for r in res.results)
    return np.float32(total / N_FULL)
